# revision 23
# baseline (speedup 1.0000x reference)
"""Causal self-attention (B=4, T=2048, C=1024, 16 heads) on 8 trn2 NeuronCores.

Sharding: batch x head-group hybrid. Core c handles batch c//2 and head
group c%2 (8 of 16 heads). Each core computes the qkv projection for its
head group over its batch's tokens, runs causal attention for its 8
heads, and produces a partial c_proj output (contraction over its 512 of
the 1024 y channels). Host sums the two partials per batch, scales by
1/512, adds b_proj.

Projections run as fp8e4m3 DoubleRow matmuls (0.5 cyc/row, contracting
2x128 channels per instruction) with first-order error compensation;
attention stays bf16 (fp8's ~4% element error is a direct y-relative
error there and would blow the 2e-2 budget, while the compensated
projections measure ~4.5e-3 end to end):
  qkv = Wh.T xh + Wl.T xh + Wh.T xl     (dropped Wl.T xl term ~0.2%)
where xh = fp8(x), xl = fp8(x - xh), Wh = fp8(32 W), Wl = fp8(32W - Wh)
are split on the host (x also pre-transposed on the host, so no DMA
transposes at all). Each 3-pass projection piece costs 0.75x its bf16
version; the c_proj runs the same way over head-pair pairs, with y's
fp8 split (16y = y8 + dy8) two DVE ops per (pair, chunk).

All rescaling is free, folded into existing ops:
  Q^T,K^T store 32(xW+b)    exp scale absorbs the 1/1024 (0.125/1024)
  V' stores 32(xW+b)        the ones column is 32 so P@V' denominators
                            carry the same factor and it cancels
  y stores 16 y             selab/e65 selector values 16 and 1/16 fold
                            the factor into the reciprocal broadcast
  out stores 512 out        host multiplies the gathered partials by
                            1/512 before adding b_proj
Data layout per core (j head-major):
  xh,xl [c, ct, tok] fp8    host-transposed, ct channel tiles of 128
  S^T [k_tok, q]            = K^T_tile.T Q^T; both heads of a pair land
                            in one 2-bank PSUM tile so a single ScalarE
                            exp covers them
  P = exp(S^T/8192) bf16    causal diagonal blocks masked by a triu
                            multiply; fully-masked columns never computed
  O' [65, q] = V'.T P       accumulated over k tiles (col 64 = denoms)
  y [128, tok] bf16 (x16)   head B's rows shifted into partitions 64..
                            by a SBUF->SBUF DMA (or the PE on the tail)
  out partial [tok, C]      3-pass DoubleRow over pair-pairs, bf16 x512

Scheduling notes (cost-model-driven):
  - Weights+x arrive partition-major (128-256 fat descriptors per DMA);
    the startup-critical chunk-0 tensors go first (x tokens 0:512 and
    the qkv weights, hi before lo), later token quarters of x after.
  - The PE p-state resolves at SEQ-visit time with ramp measured from
    the start of the engine's current continuous-execution run; a
    drained engine means the next ~3us of matmuls are charged
    0.65-1.2GHz. Dependency-free dummy matmuls on resident constants
    bridge the startup DMA wait and the last normalization chain.
  - Chunk 0 runs as two pass-major groups of 6 pieces each (V tt0-3 +
    Q0 + K4 first — exactly what pair-0 attention needs) so the matmul
    stream chases the DMA arrival order hi-x, hi-W, lo-x, lo-W without
    head-of-line blocking on the in-order PE queue.
  - Attention loops qc-outer; deferred work (next chunk's qkv pieces,
    normalization finishes) is flushed evenly across attention windows:
    attention alone is ScalarE-bound (~1040ns vs ~870ns PE per k-tile),
    so the deferred pieces are what keep PE busy. Normalization
    finishes are held back ~10 k-tiles so the PE never waits on their
    sums-DMA round trip.
  - Output projections for chunks 0-2 are deferred into chunk 3's
    attention windows: chunk 3 has no successor qkv pieces, so those
    windows are otherwise ScalarE-bound with an idle PE, while the
    earlier windows are already PE-bound. Chunk-2 projections unlock
    only after chunk-2 pair-3's normalization finish (emission order
    defines the dependency graph).
  - The last chunk's projection runs as pairs-01 partials (stashed bf16
    during pairs 2-3's attention) plus a pairs-23 finish whose stash
    re-add goes through the PE (identity matmul); the last pair
    normalizes without any cross-partition DMA: ScalarE stages the
    denominator row, the PE broadcasts it (e65 selector) and shifts
    head B (shift64), DVE takes a full-tile reciprocal. Final stores
    are bf16 and split across both HWDGE queues.
Measured end-to-end relative error vs the fp32 reference: ~4.5e-3.
"""

from contextlib import ExitStack

import numpy as np
import ml_dtypes

import concourse.bass as bass
import concourse.mybir as mybir
import concourse.tile as tile
from concourse import bacc
from concourse.bass_utils import run_bass_kernel_spmd
from concourse.masks import make_identity

F32 = mybir.dt.float32
BF16 = mybir.dt.bfloat16
F8 = mybir.dt.float8e4
DR = mybir.MatmulPerfMode.DoubleRow
F8NP = ml_dtypes.float8_e4m3

T = 2048
C = 1024
NH_LOC = 8          # heads per core
HD = 64
J = NH_LOC * HD     # 512 local q/k/v channels
N_CORES = 8
QC = 4              # q chunks of 512
TOK_TILES = 16      # token tiles of 128
CP = 4              # DoubleRow channel-tile pairs over C
PAIRS = 4           # head pairs per core
WSCALE = 32.0       # fp8 pre-scale on all weights
EXP_SCALE = 0.125 / (WSCALE * WSCALE)   # absorbs Q,K storage scale
YSCALE = 16.0       # y storage scale (via selab/e65 selector values)


def build_nc(debug_taps=False):
    nc = bacc.Bacc("TRN2", target_bir_lowering=False, debug=False)
    dbg = {}
    if debug_taps:
        dbg["qt"] = nc.dram_tensor("dbg_qt", [PAIRS * 128, T], BF16,
                                   kind="ExternalOutput")
        dbg["kt"] = nc.dram_tensor("dbg_kt", [PAIRS * 128, T], BF16,
                                   kind="ExternalOutput")
        dbg["v"] = nc.dram_tensor("dbg_v", [128, TOK_TILES * NH_LOC * 65],
                                  BF16, kind="ExternalOutput")

    xh_d = nc.dram_tensor("xh", [128, 2 * CP, T], F8, kind="ExternalInput")
    xl_d = nc.dram_tensor("xl", [128, 2 * CP, T], F8, kind="ExternalInput")
    wqh_d = nc.dram_tensor("wqh", [128, 4, CP, 2, 128], F8,
                           kind="ExternalInput")
    wql_d = nc.dram_tensor("wql", [128, 4, CP, 2, 128], F8,
                           kind="ExternalInput")
    wkh_d = nc.dram_tensor("wkh", [128, 4, CP, 2, 128], F8,
                           kind="ExternalInput")
    wkl_d = nc.dram_tensor("wkl", [128, 4, CP, 2, 128], F8,
                           kind="ExternalInput")
    wvh_d = nc.dram_tensor("wvh", [128, CP, 2, J], F8, kind="ExternalInput")
    wvl_d = nc.dram_tensor("wvl", [128, CP, 2, J], F8, kind="ExternalInput")
    bqk_d = nc.dram_tensor("bqk", [128, 8], F32, kind="ExternalInput")
    bv_d = nc.dram_tensor("bv", [J], BF16, kind="ExternalInput")
    wph_d = nc.dram_tensor("wph", [128, 2, 2, C], F8, kind="ExternalInput")
    wpl_d = nc.dram_tensor("wpl", [128, 2, 2, C], F8, kind="ExternalInput")
    out_d = nc.dram_tensor("out", [T, C], BF16, kind="ExternalOutput")

    with tile.TileContext(nc) as tc, ExitStack() as ctx:
        const = ctx.enter_context(tc.tile_pool(name="const", bufs=1))
        wpool = ctx.enter_context(tc.tile_pool(name="w", bufs=1))
        qkv = ctx.enter_context(tc.tile_pool(name="qkv", bufs=1))
        ypool = ctx.enter_context(tc.tile_pool(name="y", bufs=1))
        wk = ctx.enter_context(tc.tile_pool(name="wk", bufs=1))

        # ---- resident weights (fp8 hi/lo) and x^T (host-transposed) ----
        bqk_sb = const.tile([128, 8], F32)
        bv_sb = const.tile([1, J], BF16)
        xh = qkv.tile([128, 2 * CP, T], F8, name="xh")
        xl = qkv.tile([128, 2 * CP, T], F8, name="xl")
        wvh = wpool.tile([128, CP, 2, J], F8, name="wvh")
        wvl = wpool.tile([128, CP, 2, J], F8, name="wvl")
        wqh = wpool.tile([128, 4, CP, 2, 128], F8, name="wqh")
        wql = wpool.tile([128, 4, CP, 2, 128], F8, name="wql")
        wkh = wpool.tile([128, 4, CP, 2, 128], F8, name="wkh")
        wkl = wpool.tile([128, 4, CP, 2, 128], F8, name="wkl")
        wph = wpool.tile([128, 2, 2, C], F8, name="wph")
        wpl = wpool.tile([128, 2, 2, C], F8, name="wpl")

        # ---- constants (emitted first: no DMA dependencies, and the
        # dummy warm-up matmuls need z65/triu2/ident resident asap) ----
        ones_row = const.tile([1, 128], BF16)
        nc.vector.memset(ones_row, 1.0)
        z65 = const.tile([128, 1024], F32)
        nc.vector.memset(z65, 0.0)
        # bf16 identity: folds the last chunk's stashed partial projection
        # back into PSUM via the PE (no DVE adds on the critical tail)
        ident_bf = const.tile([128, 128], BF16)
        make_identity(nc, ident_bf)
        # triu2[p, c, f] = 1 iff f >= p, duplicated over c: masks the causal
        # diagonal 128-block of both heads' P in one tensor_tensor op.
        triu2 = const.tile([128, 2, 128], BF16)
        nc.gpsimd.memset(triu2, 0.0)
        nc.gpsimd.affine_select(
            out=triu2, in_=triu2, compare_op=mybir.AluOpType.is_gt,
            fill=1.0, base=0, pattern=[[0, 2], [-1, 128]],
            channel_multiplier=1)
        # selab[p, f] = YSCALE iff f in [64p, 64p+64): head selector for
        # the reciprocal broadcast matmul; its value folds y's storage
        # scale into the normalization for free (partition-1 memsets are
        # illegal, hence the [2,128] shape).
        selab = const.tile([2, 128], F32)
        nc.gpsimd.memset(selab, YSCALE)
        nc.gpsimd.affine_select(
            out=selab, in_=selab, compare_op=mybir.AluOpType.is_ge,
            fill=0.0, base=0, pattern=[[1, 128]], channel_multiplier=-64)
        nc.gpsimd.affine_select(
            out=selab, in_=selab, compare_op=mybir.AluOpType.is_ge,
            fill=0.0, base=63, pattern=[[-1, 128]], channel_multiplier=64)
        selab_r = const.tile([2, 128], mybir.dt.float32r)
        nc.vector.tensor_copy(selab_r, selab)
        stgr65 = const.tile([128, 1024], mybir.dt.float32r)
        nc.vector.tensor_copy(stgr65, z65)
        # e65[p, f] = 1/YSCALE iff p == 64: broadcasts the denominator row
        # of the last group's O' accumulator to all 128 partitions via the
        # PE, skipping the cross-partition sums DMA on the exposed tail
        # chain. Row 64 arrives via a late DMA (stride-0 affine_select
        # patterns misbehave on real GpSimd hardware); e65 is only read at
        # the end.
        ones_f = const.tile([1, 128], F32)
        nc.vector.memset(ones_f, 1.0 / YSCALE)
        e65 = const.tile([128, 128], F32)
        nc.vector.memset(e65, 0.0)
        # shift64[p, f] = 1 iff f == p + 64: moves the last group's head-B
        # O' rows into partitions 64..128 through the PE instead of the
        # SBUF->SBUF DMA on the exposed tail chain.
        shift64 = const.tile([64, 128], BF16)
        nc.gpsimd.memset(shift64, 1.0)
        nc.gpsimd.affine_select(
            out=shift64, in_=shift64, compare_op=mybir.AluOpType.is_ge,
            fill=0.0, base=-64, pattern=[[1, 128]], channel_multiplier=-1)
        nc.gpsimd.affine_select(
            out=shift64, in_=shift64, compare_op=mybir.AluOpType.is_ge,
            fill=0.0, base=64, pattern=[[-1, 128]], channel_multiplier=1)

        # DMA order is startup-critical: chunk 0 wants x tokens 0:512 and
        # the projection weights, hi before lo; x's later token quarters
        # follow (chunk qc's pieces run during chunk qc-1's attention,
        # far behind these).
        # Three DMA queues in parallel (each sustains only ~2 in flight,
        # and an engine's SEQ is held until its DMA clears HWDGE): x +
        # chunk-0 criticals on SP, hi weights on ScalarE (free again well
        # before its first finish copies), lo weights + late x quarters +
        # wp on the GpSimd SWDGE path, which is otherwise idle until the
        # chunk-3 stores.
        nc.sync.dma_start(xh[:, :, 0:512], xh_d[:, :, 0:512])
        nc.scalar.dma_start(wvh, wvh_d[:, :, :, :])
        nc.sync.dma_start(bqk_sb, bqk_d[:, :])
        nc.sync.dma_start(bv_sb, bv_d[:].rearrange("(a n) -> a n", a=1))
        nc.scalar.dma_start(wqh, wqh_d[:, :, :, :, :])
        nc.scalar.dma_start(wkh, wkh_d[:, :, :, :, :])
        nc.sync.dma_start(xl[:, :, 0:512], xl_d[:, :, 0:512])
        nc.gpsimd.dma_start(wvl, wvl_d[:, :, :, :])
        nc.gpsimd.dma_start(wql, wql_d[:, :, :, :, :])
        nc.gpsimd.dma_start(wkl, wkl_d[:, :, :, :, :])

        # ---- persistent activations ----
        qt_sb = [qkv.tile([128, T], BF16, name=f"qt{p}") for p in range(PAIRS)]
        kt_sb = [qkv.tile([128, T], BF16, name=f"kt{p}") for p in range(PAIRS)]
        # v layout [128, tt, head, 65]: col 64 = WSCALE so that P@V' also
        # emits softmax denominators carrying V's storage scale (cancels)
        v_sb = qkv.tile([128, TOK_TILES, NH_LOC, 65], BF16, name="v")
        nc.vector.memset(v_sb[:, :, :, 64:65], WSCALE)
        # y only lives as fp8 hi/lo (the c_proj inputs); the bf16 16y is a
        # short-lived per-(pair, chunk) stage buffer rotated in-place
        y8_sb = ypool.tile([128, PAIRS, T], F8, name="y8")
        dy8_sb = ypool.tile([128, PAIRS, T], F8, name="dy8")

        # later token quarters of x + remaining weights + the e65 row DMA
        for q0 in (512, 1024, 1536):
            nc.gpsimd.dma_start(xh[:, :, q0:q0 + 512], xh_d[:, :, q0:q0 + 512])
            nc.gpsimd.dma_start(xl[:, :, q0:q0 + 512], xl_d[:, :, q0:q0 + 512])
        nc.gpsimd.dma_start(wph, wph_d[:, :, :, :])
        nc.gpsimd.dma_start(wpl, wpl_d[:, :, :, :])
        nc.sync.dma_start(e65[64:65, :], ones_f[0:1, :])
        e65_r = const.tile([128, 128], mybir.dt.float32r)
        nc.vector.tensor_copy(e65_r, e65)

        # ====== fused pipeline: qkv projection chunks overlap attention ====
        # One PSUM pool for the whole kernel (8 banks):
        #   S    [128,1024] x2  exp-pipeline score tiles            4 banks
        #   O    [.,512]    x2  O' accumulators / chunk-0 accs      2 banks
        #   acc  [128,512]  x1  qkv-projection + c_proj accums      1 bank
        #   bcpo [128,512]  x1  recip broadcasts + c_proj           1 bank
        # Chunk 0 runs before attention and borrows the idle S/O banks for
        # 6 parallel pass-major accumulators (s1 is reserved for the
        # p-state warm-up dummies).
        with tc.tile_pool(name="ps", bufs=1, space="PSUM") as psb:
            pending = []
            late = [[] for _ in range(QC)]   # deferred c_proj per chunk
            late_ready = []
            late2 = []

            def try_flush(allow_fin):
                """Run one deferred item. When the queue front is a
                held-back normalization finish, skip ahead to the first
                reorderable piece (qkv pieces touch only qt/kt/v — data-
                independent of every fin) or a ready c_proj piece, so the
                hold never head-of-line-blocks the PE filler stream.
                Non-reorderable items (proj3 stash/fin3) are never jumped
                ahead of the fins they depend on: emission order defines
                the dependency graph."""
                fn = None
                if pending and (allow_fin or
                                not getattr(pending[0], "is_norm_fin",
                                            False)):
                    fn = pending.pop(0)
                else:
                    for i, f in enumerate(pending):
                        if getattr(f, "reorderable", False):
                            fn = pending.pop(i)
                            break
                    if fn is None and late_ready:
                        fn = late_ready.pop(0)
                if fn is None:
                    return False
                fn()
                if getattr(fn, "unlocks_late2", False):
                    late_ready.extend(late2)
                    late2.clear()
                return True

            def n_deferred():
                return len(pending) + len(late_ready) + len(late2)

            def flush_all():
                while n_deferred():
                    if not try_flush(True):
                        # unlock fin never materialized (defensive)
                        late_ready.extend(late2)
                        late2.clear()

            def wqk_views(jt, cp):
                if jt < 4:
                    return wqh[:, jt, cp], wql[:, jt, cp]
                return wkh[:, jt - 4, cp], wkl[:, jt - 4, cp]

            def qk_finish(jt, qc, pm):
                # late chunks run during ScalarE-heavy attention: use DVE.
                # Chunk 0 alternates engines so its finish chain (which
                # gates the accumulator reuse and first attention pair)
                # isn't serial on one engine.
                dst = qt_sb[jt] if jt < 4 else kt_sb[jt - 4]
                if qc >= 2 or (qc == 0 and jt % 2 == 1):
                    nc.vector.tensor_scalar_add(
                        dst[:, qc * 512:(qc + 1) * 512], pm,
                        bqk_sb[:, jt:jt + 1])
                else:
                    nc.scalar.activation(
                        dst[:, qc * 512:(qc + 1) * 512], pm,
                        mybir.ActivationFunctionType.Identity,
                        bias=bqk_sb[:, jt:jt + 1])

            def v_finish(tta, pv):
                nc.tensor.matmul(pv, ones_row, bv_sb, start=False, stop=True)
                src = pv.rearrange("p (h w) -> p h w", h=NH_LOC)
                if tta >= 8 or (tta < 4 and tta % 2 == 1):
                    nc.vector.tensor_copy(v_sb[:, tta, :, 0:64], src)
                else:
                    nc.scalar.copy(v_sb[:, tta, :, 0:64], src)

            # compensated-fp8 pass list: hi@hi, hi-W@lo-x, lo-W@hi-x
            PASSES = ("hh", "hl", "lh")

            def dr_qk(pm, jt, q0, qn, pi, cp, start, stop):
                wh, wl = wqk_views(jt, cp)
                w = wl if PASSES[pi] == "lh" else wh
                xsrc = xl if PASSES[pi] == "hl" else xh
                nc.tensor.matmul(pm, w, xsrc[:, 2 * cp:2 * cp + 2, q0:q0 + qn],
                                 start=start, stop=stop, perf_mode=DR)

            def dr_v(pv, tta, pi, cp, start):
                w = wvl[:, cp] if PASSES[pi] == "lh" else wvh[:, cp]
                xsrc = xl if PASSES[pi] == "hl" else xh
                nc.tensor.matmul(
                    pv, xsrc[:, 2 * cp:2 * cp + 2, tta * 128:(tta + 1) * 128],
                    w, start=start, stop=False, perf_mode=DR)

            # ---- chunk 0 ----
            # The cost model resolves the PE p-state at SEQ-visit time with
            # ramp measured from the start of the engine's current
            # continuous-execution run: a drained engine means the next
            # ~3us of matmuls execute at 0.65-1.2GHz. A burst of cheap
            # dependency-free dummy matmuls (consts -> the s1 bank) keeps
            # the run alive until the first x/weight DMAs land, so every
            # real matmul (visited only when its semaphore fires, with the
            # engine long-busy) is charged full speed.
            a0 = psb.tile([128, 512], F32, tag="acc", bufs=1, name="a0")
            a1 = psb.tile([128, 512], F32, tag="bcpo", bufs=1, name="a1")
            s0 = psb.tile([128, 1024], F32, tag="S", bufs=2, name="s0")
            s1 = psb.tile([128, 1024], F32, tag="S", bufs=2, name="s1")
            o0 = psb.tile([128, 512], F32, tag="O", bufs=2, name="o0")
            o1 = psb.tile([128, 512], F32, tag="O", bufs=2, name="o1")
            triu_flat = triu2.rearrange("p a b -> p (a b)")

            def dummies(n):
                # dependency-free p-state keep-alives: the cost model
                # charges 0.65-1.2GHz for ~3us after any PE drain, so
                # bridging DMA-paced waits with cheap matmuls keeps the
                # real ones at full speed.
                for _ in range(n):
                    nc.tensor.matmul(s1[:, 0:256], ident_bf, triu_flat,
                                     start=True, stop=True)

            # the very first keep-alives only need ones_row (a single DVE
            # memset), starting the PE run ~1.2us before the identity/triu
            # constants exist
            for _ in range(30):
                nc.tensor.matmul(s1[:, 0:128], ones_row, ones_row,
                                 start=True, stop=True)
            dummies(22)
            # group 1 (pass-major): V tt0-3 + Q jt0 + K jt4 — exactly the
            # pieces pair-0's attention needs, in six accumulators. Dummy
            # bursts between passes bridge each pass's DMA arrival wait.
            vacc = [a0, a1, s0[:, 0:512], s0[:, 512:1024]]
            for pi in range(3):
                for cp in range(CP):
                    for tt in range(4):
                        dr_v(vacc[tt], tt, pi, cp,
                             start=(pi == 0 and cp == 0))
                    for jt, pm in ((0, o0), (4, o1)):
                        dr_qk(pm, jt, 0, 512, pi, cp,
                              start=(pi == 0 and cp == 0),
                              stop=(pi == 2 and cp == CP - 1))
                dummies(10 if pi < 2 else 6)
            for tt in range(4):
                v_finish(tt, vacc[tt])
            qk_finish(0, 0, o0)
            qk_finish(4, 0, o1)
            # group 2 (pass-major): remaining Q/K pieces in the now-free
            # accumulators.
            g2 = [(1, a0), (5, a1), (2, s0[:, 0:512]), (6, s0[:, 512:1024]),
                  (3, o0), (7, o1)]
            for pi in range(3):
                for cp in range(CP):
                    for jt, pm in g2:
                        dr_qk(pm, jt, 0, 512, pi, cp,
                              start=(pi == 0 and cp == 0),
                              stop=(pi == 2 and cp == CP - 1))
            for jt, pm in g2:
                qk_finish(jt, 0, pm)

            # qkv pieces alternate between the acc and bcpo banks so a
            # piece's accumulation never stalls on the previous piece's
            # PSUM->SBUF read (2-deep software pipeline).
            piece_tag = [0]

            def next_tag():
                piece_tag[0] ^= 1
                return "acc" if piece_tag[0] else "bcpo"

            def a_pieces(qc):
                """Emit-later closures computing Q^T/K^T, V' for chunk qc."""
                pieces = []

                def qk_piece(jt):
                    def run():
                        pm = psb.tile([128, 512], F32, tag=next_tag(),
                                      bufs=1, name="pm")
                        for pi in range(3):
                            for cp in range(CP):
                                dr_qk(pm, jt, qc * 512, 512, pi, cp,
                                      start=(pi == 0 and cp == 0),
                                      stop=(pi == 2 and cp == CP - 1))
                        qk_finish(jt, qc, pm)
                    return run

                def v_piece(tt):
                    def run():
                        tta = qc * 4 + tt
                        pv = psb.tile([128, J], F32, tag=next_tag(),
                                      bufs=1, name="pv")
                        for pi in range(3):
                            for cp in range(CP):
                                dr_v(pv, tta, pi, cp,
                                     start=(pi == 0 and cp == 0))
                        v_finish(tta, pv)
                    return run

                for jt in range(8):
                    pieces.append(qk_piece(jt))
                for tt in range(4):
                    pieces.append(v_piece(tt))
                return pieces

            def split_y(p, q0, ystg):
                """fp8 hi/lo split of the normalized (16x) y for c_proj."""
                nc.vector.tensor_copy(y8_sb[:, p, q0:q0 + 512], ystg)
                nc.vector.tensor_tensor(
                    dy8_sb[:, p, q0:q0 + 512], ystg,
                    y8_sb[:, p, q0:q0 + 512], mybir.AluOpType.subtract)

            def emit_tail(p, q0, o_a, o_b, last=False, unlock=False):
                ystg = wk.tile([128, 512], BF16, tag="ystg", bufs=3)
                if last:
                    # exposed tail: sums copies go first on the now-idle
                    # ScalarE (they feed the longest chain) while DVE moves
                    # the O' rows; head B goes through the PE (shift64)
                    # instead of the SBUF->SBUF DMA, and the denominator
                    # row is broadcast via the PE + full-tile reciprocal
                    # (single-partition reciprocal at offset 64 silently
                    # returns zeros on real hardware). Dummy matmuls keep
                    # the PE run alive across the chain so the projection
                    # finish is charged full speed.
                    stg_s = wk.tile([65, 1024], F32, tag="stgs", bufs=2)
                    nc.scalar.copy(stg_s[64:65, 0:512], o_a[64:65, :])
                    nc.scalar.copy(stg_s[64:65, 512:1024], o_b[64:65, :])
                    nc.vector.tensor_copy(stgr65[64:65, :],
                                          stg_s[64:65, :])
                    stg_b = wk.tile([64, 512], BF16, tag="stgb", bufs=2)
                    nc.vector.tensor_copy(stg_b, o_b[0:64, :])
                    nc.vector.tensor_copy(ystg[0:64, :], o_a[0:64, :])
                    for _ in range(4):
                        nc.tensor.matmul(s1[:, 0:512], z65[:, 0:128],
                                         z65[:, 512:1024],
                                         start=True, stop=True)

                    def fin():
                        # bc2[p, :] = (sum_a | sum_b) for every p: head A
                        # rows read the left half, head B rows the right
                        bc2 = psb.tile([128, 1024], F32, tag="S", bufs=2,
                                       name="bc2")
                        nc.tensor.matmul(bc2[:, 0:512], e65_r,
                                         stgr65[:, 0:512],
                                         start=True, stop=True)
                        nc.tensor.matmul(bc2[:, 512:1024], e65_r,
                                         stgr65[:, 512:1024],
                                         start=True, stop=True)
                        yb = psb.tile([128, 512], F32, tag="bcpo", bufs=1,
                                      name="yb")
                        nc.tensor.matmul(yb, shift64, stg_b,
                                         start=True, stop=True)
                        recs = wk.tile([128, 1024], F32, tag="ob3", bufs=4)
                        nc.vector.reciprocal_approx_fast(recs, bc2)
                        nc.vector.tensor_mul(ystg[0:64, :], ystg[0:64, :],
                                             recs[0:64, 0:512])
                        nc.vector.tensor_mul(ystg[64:128, :], yb[64:128, :],
                                             recs[64:128, 512:1024])
                        split_y(p, q0, ystg)
                    pending.append(fin)
                    return
                # head A rows land aligned; stage sums + head B rows
                nc.vector.tensor_copy(ystg[0:64, :], o_a[0:64, :])
                stg_b = wk.tile([64, 512], BF16, tag="stgb", bufs=2)
                nc.vector.tensor_copy(stg_b, o_b[0:64, :])
                stg_s = wk.tile([65, 1024], F32, tag="stgs", bufs=2)
                nc.vector.tensor_copy(stg_s[64:65, 0:512], o_a[64:65, :])
                nc.vector.tensor_copy(stg_s[64:65, 512:1024], o_b[64:65, :])
                nc.sync.dma_start(ystg[64:128, :], stg_b)
                sums = wk.tile([2, 512], F32, tag="sums", bufs=3)
                nc.sync.dma_start(sums[0:1, :], stg_s[64:65, 0:512])
                nc.sync.dma_start(sums[1:2, :], stg_s[64:65, 512:1024])
                rec = wk.tile([2, 512], F32, tag="rec", bufs=3)
                nc.vector.reciprocal_approx_fast(rec, sums)
                # f32r matmul inputs must come from a rounding producer
                rec_r = wk.tile([2, 512], mybir.dt.float32r, tag="recr",
                                bufs=2)
                nc.vector.tensor_copy(rec_r, rec)

                def fin():
                    # f32r runs 1 cyc/row vs fp32's 4 (values are exact
                    # selector entries times fp32 reciprocals; f32r's
                    # reduced multiply precision is irrelevant here)
                    bc = psb.tile([128, 512], F32, tag="bcpo", bufs=1,
                                  name="bc")
                    nc.tensor.matmul(bc, selab_r, rec_r,
                                     start=True, stop=True)
                    nc.vector.tensor_mul(ystg, ystg, bc)
                    split_y(p, q0, ystg)
                fin.is_norm_fin = True
                fin.unlocks_late2 = unlock
                pending.append(fin)

            def dr_proj(po, pp, tt, oc, pi, start, stop):
                lhs = dy8_sb if PASSES[pi] == "hl" else y8_sb
                w = wpl if PASSES[pi] == "lh" else wph
                nc.tensor.matmul(
                    po, lhs[:, 2 * pp:2 * pp + 2, tt * 128:(tt + 1) * 128],
                    w[:, pp, :, oc * 512:(oc + 1) * 512],
                    start=start, stop=stop, perf_mode=DR)

            def make_proj_piece(tt, oc):
                # one output-projection piece for token tile tt (needs all
                # 4 pairs' y8/dy8 for tt): 2 pair-pairs x 3 passes
                def proj():
                    po = psb.tile([128, 512], F32, tag=next_tag(), bufs=1,
                                  name="po")
                    for pp in range(2):
                        for pi in range(3):
                            dr_proj(po, pp, tt, oc, pi,
                                    start=(pp == 0 and pi == 0),
                                    stop=(pp == 1 and pi == 2))
                    ob = wk.tile([128, 512], BF16, tag="ob", bufs=3)
                    nc.vector.tensor_copy(ob, po)
                    nc.gpsimd.dma_start(
                        out_d[tt * 128:(tt + 1) * 128,
                              oc * 512:(oc + 1) * 512], ob)
                return proj

            proj3_stash = {}

            def make_proj3_part(tt, oc):
                # pairs 0-1 of the last chunk's projection, stashed to SBUF
                # as bf16 so the tail can re-add it through the PE
                def part():
                    po = psb.tile([128, 512], F32, tag=next_tag(), bufs=1,
                                  name="po3p")
                    for pi in range(3):
                        dr_proj(po, 0, tt, oc, pi,
                                start=(pi == 0), stop=(pi == 2))
                    st = wk.tile([128, 512], BF16, tag="stash", bufs=8)
                    nc.vector.tensor_copy(st, po)
                    proj3_stash[(tt, oc)] = st
                return part

            def make_proj3_fin(tt):
                # pairs-23 term + PE-folded stash add, one piece per token
                # tile; copies alternate ScalarE/DVE, stores alternate the
                # two HWDGE queues (each sustains only ~2 DMAs in flight)
                def fin3():
                    po = psb.tile([128, 1024], F32, tag="S", bufs=2,
                                  name="po3f")
                    for oc in range(2):
                        nc.tensor.matmul(
                            po[:, oc * 512:(oc + 1) * 512],
                            ident_bf, proj3_stash[(tt, oc)],
                            start=True, stop=False)
                        for pi in range(3):
                            dr_proj(po[:, oc * 512:(oc + 1) * 512], 1, tt,
                                    oc, pi, start=False, stop=(pi == 2))
                    # half on each engine: the copies pace the exposed
                    # tail, not the PE folds
                    ob = wk.tile([128, 1024], BF16, tag="ob3", bufs=4)
                    nc.scalar.copy(ob[:, 0:512], po[:, 0:512])
                    nc.vector.tensor_copy(ob[:, 512:1024], po[:, 512:1024])
                    if tt % 2:
                        nc.scalar.dma_start(
                            out_d[tt * 128:(tt + 1) * 128, :], ob)
                    else:
                        nc.sync.dma_start(
                            out_d[tt * 128:(tt + 1) * 128, :], ob)
                return fin3

            a_left = [0] * QC  # un-flushed A pieces per chunk

            def count_piece(piece, qc):
                def run():
                    a_left[qc] -= 1
                    piece()
                run.reorderable = True
                return run

            for qc in range(QC):
                q0 = qc * 512
                n_kt = 4 * (qc + 1)
                if qc + 1 < QC:
                    pcs = a_pieces(qc + 1)
                    a_left[qc + 1] = len(pcs)
                    pending.extend(count_piece(pc, qc + 1) for pc in pcs)
                if qc == QC - 1:
                    # chunk 3's windows are the ScalarE-bound ones with no
                    # successor qkv pieces: release the deferred c_proj
                    # work there. Chunk-2 pieces wait for chunk-2 pair-3's
                    # normalization finish (the unlock fin).
                    late_ready.extend(late[0])
                    late_ready.extend(late[1])
                    late2.extend(late[2])
                # emission barrier: attention for qc depends on chunk qc's
                # Q/K/V writes being *emitted* (Tile tracks deps in trace
                # order); normally a no-op since pieces drain during qc-1.
                while a_left[qc] > 0:
                    try_flush(True)
                for p in range(PAIRS):
                    o_a = psb.tile([65, 512], F32, tag="O", bufs=2, name="o_a")
                    o_b = psb.tile([65, 512], F32, tag="O", bufs=2, name="o_b")
                    staged = {}

                    def emit_s(kt):
                        off = max(0, kt * 128 - q0)
                        # S for both heads in one 2-bank psum tile so one
                        # ScalarE exp covers both
                        s_ab = psb.tile([128, 1024], F32, tag="S", bufs=2,
                                        name="s_ab")
                        for half in range(2):
                            r0, r1 = half * 64, half * 64 + 64
                            nc.tensor.matmul(
                                s_ab[:, half * 512 + off:half * 512 + 512],
                                kt_sb[p][r0:r1, kt * 128:(kt + 1) * 128],
                                qt_sb[p][r0:r1, q0 + off:q0 + 512],
                                start=True, stop=True)
                        staged[kt] = (s_ab, off)

                    def emit_consume(kt):
                        s_ab, off = staged.pop(kt)
                        p_ab = wk.tile([128, 1024], BF16, tag="P", bufs=5,
                                       name="p_ab")
                        s3 = s_ab.rearrange("p (c w) -> p c w", c=2)
                        p3 = p_ab.rearrange("p (c w) -> p c w", c=2)
                        nc.scalar.activation(
                            p3[:, :, off:512], s3[:, :, off:512],
                            mybir.ActivationFunctionType.Exp,
                            scale=EXP_SCALE)
                        if kt * 128 >= q0:  # causal diagonal block
                            nc.vector.tensor_mul(
                                p3[:, :, off:off + 128],
                                p3[:, :, off:off + 128], triu2)
                        first, last = (kt == 0), (kt == n_kt - 1)
                        nc.tensor.matmul(o_a[:, off:512],
                                         v_sb[:, kt, 2 * p, :],
                                         p_ab[:, off:512],
                                         start=first, stop=last)
                        nc.tensor.matmul(o_b[:, off:512],
                                         v_sb[:, kt, 2 * p + 1, :],
                                         p_ab[:, 512 + off:1024],
                                         start=first, stop=last)

                    # software pipeline: emit S(kt) one step ahead of its
                    # exp/mask/PV consumers so PE never waits for ScalarE.
                    # Pending PE-only pieces are spread evenly across the
                    # remaining attention windows of this chunk: attention
                    # alone is ScalarE-bound, so the pieces are what keep
                    # PE busy.
                    quota = -(-n_deferred() // (PAIRS - p))
                    hold = min(10, n_kt)
                    # the very last window stretches its flush schedule so
                    # a few c_proj pieces remain to feed the PE during the
                    # final exps and the exposed normalization chain
                    den = n_kt + (3 if qc == QC - 1 and p == PAIRS - 1
                                  else 0)
                    flushed = 0
                    for kt in range(n_kt + 1):
                        if kt < n_kt:
                            emit_s(kt)
                        if kt >= 1:
                            target = quota * kt // den
                            while flushed < target:
                                # hold a normalization finish back ~10
                                # k-tiles so PE never waits on its
                                # sums-DMA round trip (~5us); reorderable
                                # qkv pieces and ready c_proj pieces fill
                                # in behind the hold instead
                                if try_flush(kt >= hold):
                                    flushed += 1
                                else:
                                    break
                            emit_consume(kt - 1)
                    if qc == QC - 1 and p == PAIRS - 1:
                        # emit the held-back pieces now: they execute on
                        # the PE during the final exps and the exposed
                        # normalization chain
                        while try_flush(False):
                            pass
                    emit_tail(p, q0, o_a, o_b,
                              last=(qc == QC - 1 and p == PAIRS - 1),
                              unlock=(qc == 2 and p == PAIRS - 1))
                    if qc == QC - 1 and p == 1:
                        # last chunk: pairs 0-1's partial projection runs
                        # during pairs 2-3's attention so only the
                        # pairs-23 term + fold remains after pair 3.
                        for tt in range(qc * 4, qc * 4 + 4):
                            for oc in range(2):
                                pending.append(make_proj3_part(tt, oc))
                if qc == QC - 1:
                    for tt in range(qc * 4, qc * 4 + 4):
                        pending.append(make_proj3_fin(tt))
                else:
                    for tt in range(qc * 4, qc * 4 + 4):
                        for oc in range(2):
                            late[qc].append(make_proj_piece(tt, oc))
            flush_all()
            if debug_taps:
                for p in range(PAIRS):
                    nc.sync.dma_start(dbg["qt"][p * 128:(p + 1) * 128, :],
                                      qt_sb[p])
                    nc.sync.dma_start(dbg["kt"][p * 128:(p + 1) * 128, :],
                                      kt_sb[p])
                nc.sync.dma_start(
                    dbg["v"], v_sb.rearrange("p a b c -> p (a b c)"))

    nc.compile()
    return nc


_NC_CACHE = {}


def _get_nc():
    if "nc" not in _NC_CACHE:
        _NC_CACHE["nc"] = build_nc()
    return _NC_CACHE["nc"]


def _split8(a, scale):
    """Scale then split into fp8e4m3 hi + lo (first-order residual)."""
    a = np.asarray(a, dtype=np.float32) * scale
    hi = a.astype(F8NP)
    lo = (a - hi.astype(np.float32)).astype(F8NP)
    return np.ascontiguousarray(hi), np.ascontiguousarray(lo)


def shard_inputs(x, W_attn, b_attn, W_proj):
    """Per-core input maps. Core c: batch c//2, head group c%2."""
    bf = ml_dtypes.bfloat16
    x = np.asarray(x, dtype=np.float32)
    W_attn = np.asarray(W_attn, dtype=np.float32)
    b_attn = np.asarray(b_attn, dtype=np.float32)
    W_proj = np.asarray(W_proj, dtype=np.float32)
    in_maps = []
    for c in range(N_CORES):
        b, hg = c // 2, c % 2
        qs, ks, vs = hg * J, C + hg * J, 2 * C + hg * J
        # x^T [p, ct, t] with channel c = ct*128 + p, host-transposed
        xt = x[b].T.reshape(2 * CP, 128, T).transpose(1, 0, 2)
        xhh, xll = _split8(xt, 1.0)
        # wq/wk [p, jt, cp, i, jcol]: channel (2cp+i)*128 + p, j jt*128+jcol
        def qk_layout(w):
            return w.reshape(CP, 2, 128, 4, 128).transpose(2, 3, 0, 1, 4)
        wq_h, wq_l = _split8(qk_layout(W_attn[:, qs:qs + J]), WSCALE)
        wk_h, wk_l = _split8(qk_layout(W_attn[:, ks:ks + J]), WSCALE)
        wv_h, wv_l = _split8(
            W_attn[:, vs:vs + J].reshape(CP, 2, 128, J).transpose(2, 0, 1, 3),
            WSCALE)
        # bqk[p, jt]: bias for channel jt*128 + p (q for jt<4, k for jt>=4),
        # pre-scaled to match the WSCALE'd projection accumulators
        bqk = np.ascontiguousarray(
            np.concatenate([b_attn[qs:qs + J], b_attn[ks:ks + J]])
            .reshape(8, 128).T) * WSCALE
        bv = np.ascontiguousarray(b_attn[vs:vs + J] * WSCALE).astype(bf)
        wp_h, wp_l = _split8(
            W_proj[hg * J:(hg + 1) * J, :]
            .reshape(2, 2, 128, C).transpose(2, 0, 1, 3), WSCALE)
        in_maps.append({
            "xh": xhh, "xl": xll,
            "wqh": wq_h, "wql": wq_l, "wkh": wk_h, "wkl": wk_l,
            "wvh": wv_h, "wvl": wv_l,
            "bqk": np.ascontiguousarray(bqk, dtype=np.float32), "bv": bv,
            "wph": wp_h, "wpl": wp_l,
        })
    return in_maps


def kernel(x, W_attn, b_attn, W_proj, b_proj):
    nc = _get_nc()
    in_maps = shard_inputs(x, W_attn, b_attn, W_proj)
    res = run_bass_kernel_spmd(nc, in_maps, list(range(N_CORES)))
    b_proj = np.asarray(b_proj, dtype=np.float32)
    # device partials carry the YSCALE*WSCALE = 512 projection scale
    descale = 1.0 / (YSCALE * WSCALE)
    outs = []
    for b in range(4):
        partial = (np.asarray(res.results[2 * b]["out"], dtype=np.float32) +
                   np.asarray(res.results[2 * b + 1]["out"], dtype=np.float32))
        outs.append(partial * descale + b_proj[None, :])
    return np.stack(outs, axis=0)
